# revision 18
# baseline (speedup 1.0000x reference)
"""Trainium2 Bass kernel for nn_CausalDAG (gnn_message_passing).

Computation (per batch row b):
    m[b]   = A^T @ x[b]                      # concept mixing, [C, D]
    h[b,c] = ELU(W1[c] @ m[b,c] + b1[c])     # per-concept Linear(D->G)
    out[b,c] = W2[c] @ h[b,c] + b2[c]        # per-concept Linear(G->D)

Kernel strategy (pure data-parallel over batch, 8 cores):
  - Fuse step 1+2 into one dense matmul: H_pre = X @ U where
    U[(j,d),(c,g)] = A[j,c] * W1[c,g,d]  ([1024, 512], dense).
  - The host stages x pre-transposed per core as XT [F, B_CORE] so the
    contraction dim lands on SBUF partitions directly - zero on-chip
    transposes (f16 I/O: half the HBM traffic of f32).
  - ELU as h' = relu(z) + min(exp(z), 1) (= elu(z) + 1); the "+1" is
    folded into an adjusted output bias b2_eff = b2 - sum_g W2.
  - Step 3 uses h' tiles as the matmul *stationary* operand so the output
    lands directly in natural [batch, (c,d)] layout.
  - Step 3 + store run two chunks behind the fused matmul so the ELU
    chain (Scalar exp/relu -> GpSimd add) never stalls the PE.

Self-contained: hardcodes shapes; only imports the system concourse repo.
"""

import os
import sys

import numpy as np

for _p in ("/opt/trn_rl_repo", "/root/.axon_site/_ro/trn_rl_repo"):
    if os.path.isdir(_p) and _p not in sys.path:
        sys.path.insert(0, _p)
        break

import concourse.bass as bass  # noqa: E402
import concourse.bacc as bacc_mod  # noqa: E402
import concourse.mybir as mybir  # noqa: E402
import concourse.tile as tile  # noqa: E402
from concourse.bass_utils import run_bass_kernel_spmd  # noqa: E402

try:
    import ml_dtypes

    _BF16_NP = ml_dtypes.bfloat16
except ImportError:  # pragma: no cover
    _BF16_NP = None

B, C, D, G = 65536, 16, 64, 32
F = C * D  # 1024 flattened feature dim
H = C * G  # 512 hidden dim
N_CORES = 8
B_CORE = B // N_CORES  # 8192
NB = 512  # batch rows per chunk (4 row-tiles of 128)
N_CHUNKS = B_CORE // NB  # 16
NT = NB // 128  # row-tiles per chunk
KSL = F // 128  # 8 contraction slices
NQ = H // 128  # 4 concept groups of (4 concepts x 32 g)
TAIL_LAG = 1  # step-3 pipeline depth (chunks)

MM_MODE = os.environ.get("GNN_MM_MODE", "f16")


def _mm_dtypes(mode):
    if mode == "bf16":
        return mybir.dt.bfloat16, np.dtype(_BF16_NP)
    if mode == "f16":
        return mybir.dt.float16, np.dtype(np.float16)
    if mode == "f32r":
        return mybir.dt.float32r, np.dtype(np.float32)
    return mybir.dt.float32, np.dtype(np.float32)


def build_bass(mode=MM_MODE):
    mm_dt, _ = _mm_dtypes(mode)
    f32 = mybir.dt.float32
    compact = mm_dt in (mybir.dt.bfloat16, mybir.dt.float16)
    o_dt = mm_dt if compact else f32

    nc = bacc_mod.Bacc()
    UW = KSL * H  # usb (q-major: [q][k][128])
    CW = NQ * 256  # vsb
    BW = F + NQ  # b2r | b1p  (f32)
    xs = nc.declare_dram_parameter("xs", [F, B_CORE], mm_dt, isOutput=False)
    usb_d = nc.declare_dram_parameter("usb", [128, UW], mm_dt, isOutput=False)
    cstm_d = nc.declare_dram_parameter("cstm", [128, CW], mm_dt, isOutput=False)
    cstb_d = nc.declare_dram_parameter("cstb", [128, BW], f32, isOutput=False)
    out_d = nc.declare_dram_parameter("out", [B_CORE, F], o_dt, isOutput=True)

    with tile.TileContext(nc) as tc:
        with (
            tc.tile_pool(name="consts", bufs=1) as consts,
            tc.tile_pool(name="xt", bufs=4) as xt_pool,
            tc.tile_pool(name="hmat", bufs=TAIL_LAG + 1) as h_pool,
            tc.tile_pool(name="escr", bufs=2) as e_pool,
            tc.tile_pool(name="osb", bufs=2) as out_pool,
            tc.tile_pool(name="hp", bufs=4, space="PSUM") as hp_pool,
            tc.tile_pool(name="outp", bufs=4, space="PSUM") as outp_pool,
        ):
            # All consts ride the otherwise-idle output queue so the first
            # x chunk streams on the input queue concurrently.
            usb = consts.tile([128, UW], mm_dt, tag="usb")
            nc.scalar.dma_start(usb[:], usb_d[:])
            cstm = consts.tile([128, CW], mm_dt, tag="cstm")
            nc.scalar.dma_start(cstm[:], cstm_d[:])
            cstb = consts.tile([128, BW], f32, tag="cstb")
            nc.scalar.dma_start(cstb[:], cstb_d[:])
            vsb = cstm[:, 0:CW]
            b2r = cstb[:, 0:F]
            b1p = cstb[:, F:BW]

            def emit_tail(prev_ci, prev_hqs):
                # step3 + bias + store for a finished chunk, TAIL_LAG chunks
                # behind so the ELU chain latency hides under later chunks'
                # fused matmuls on PE.
                b0p = prev_ci * NB
                o_t = out_pool.tile([128, NT * F], o_dt, tag="osb")
                for t in range(NT):
                    for half in range(2):
                        op = outp_pool.tile([128, 512], f32, tag="outp")
                        for qq in range(2):
                            q = half * 2 + qq
                            nc.tensor.matmul(
                                op[:, qq * 256 : (qq + 1) * 256],
                                lhsT=prev_hqs[q][:, t * 128 : (t + 1) * 128],
                                rhs=vsb[:, q * 256 : (q + 1) * 256],
                                start=True,
                                stop=True,
                            )
                        o_slice = o_t[
                            :, t * F + half * 512 : t * F + (half + 1) * 512
                        ]
                        b2_slice = b2r[:, half * 512 : (half + 1) * 512]
                        nc.vector.scalar_tensor_tensor(
                            o_slice,
                            op[:],
                            1.0,
                            b2_slice,
                            mybir.AluOpType.mult,
                            mybir.AluOpType.add,
                        )
                    # per-row-tile store: drains right behind the bias-add
                    nc.scalar.dma_start(
                        out_d[b0p + t * 128 : b0p + (t + 1) * 128, :],
                        o_t[:, t * F : (t + 1) * F],
                    )

            def load_chunk(ci):
                b0 = ci * NB
                x_t = xt_pool.tile([128, KSL * NB], mm_dt, tag="xt")
                src = xs[:, b0 : b0 + NB].rearrange("(k p) b -> p k b", p=128)
                dst = x_t[:].rearrange("p (k b) -> p k b", k=KSL)
                nc.sync.dma_start(dst, src)
                return x_t

            PREF = 3
            xq = [load_chunk(ci) for ci in range(min(PREF, N_CHUNKS))]

            pending = []
            for ci in range(N_CHUNKS):
                if ci + PREF < N_CHUNKS:
                    xq.append(load_chunk(ci + PREF))
                x_t = xq.pop(0)

                # ---- step 3 of the chunk TAIL_LAG back (software pipeline) ----
                if len(pending) >= TAIL_LAG:
                    emit_tail(*pending.pop(0))

                # ---- fused step 1+2: H_T[q] = U_q^T @ X_T  (PSUM f32) ----
                hqs = []
                for q in range(NQ):
                    hp = hp_pool.tile([128, NB], f32, tag="hp")
                    for k in range(KSL):
                        q0 = q * KSL * 128
                        lhsT = usb[:, q0 + k * 128 : q0 + (k + 1) * 128]
                        nc.tensor.matmul(
                            hp[:],
                            lhsT=lhsT,
                            rhs=x_t[:, k * NB : (k + 1) * NB],
                            start=(k == 0),
                            stop=(k == KSL - 1),
                        )
                    # ---- ELU': h' = relu(z) + min(exp(z), 1),  z = hp + b1 ----
                    e_t = e_pool.tile([128, NB], mm_dt, tag="ee")
                    nc.scalar.activation(
                        e_t[:],
                        hp[:],
                        mybir.ActivationFunctionType.Exp,
                        bias=b1p[:, q : q + 1],
                    )
                    r_t = e_pool.tile([128, NB], mm_dt, tag="er")
                    nc.scalar.activation(
                        r_t[:],
                        hp[:],
                        mybir.ActivationFunctionType.Relu,
                        bias=b1p[:, q : q + 1],
                    )
                    h_q = h_pool.tile([128, NB], mm_dt, tag=f"h{q}")
                    nc.vector.scalar_tensor_tensor(  # h = min(e,1) + r
                        h_q[:],
                        e_t[:],
                        1.0,
                        r_t[:],
                        mybir.AluOpType.min,
                        mybir.AluOpType.add,
                    )
                    hqs.append(h_q)

                pending.append((ci, hqs))

            for args in pending:
                emit_tail(*args)

    nc.compile()
    return nc


def _host_tensors(A, W1, b1, W2, b2, mode=MM_MODE):
    _, mm_np = _mm_dtypes(mode)
    A = np.asarray(A, np.float32)
    W1 = np.asarray(W1, np.float32)
    b1 = np.asarray(b1, np.float32)
    W2 = np.asarray(W2, np.float32)
    b2 = np.asarray(b2, np.float32)

    # U[(j,d), (c,g)] = A[j,c] * W1[c,g,d]
    U = np.einsum("jc,cgd->jdcg", A, W1).reshape(F, H)
    # usb[p, q*KSL*128 + k*128 + m] = U[k*128 + p, q*128 + m]  (q-major)
    usb = np.ascontiguousarray(
        U.reshape(KSL, 128, NQ, 128).transpose(1, 2, 0, 3).reshape(128, KSL * H)
    )
    # V_q[(ct,g), (ct',d)] = delta * W2[4q+ct, d, g]; vsb[p, q*256 + n]
    vsb = np.zeros((128, NQ * 256), np.float32)
    for q in range(NQ):
        for ct in range(4):
            c = 4 * q + ct
            vsb[ct * G : (ct + 1) * G, q * 256 + ct * D : q * 256 + (ct + 1) * D] = (
                W2[c].T
            )
    b1cols = b1.reshape(H)  # [(c,g)] c-major == (q, ct, g)
    b1q = b1cols.reshape(NQ, 128).T  # [128, NQ]
    b2eff = (b2 - W2.sum(axis=2)).reshape(F)
    b2r = np.broadcast_to(b2eff, (128, F))

    cstb = np.concatenate([np.asarray(b2r, np.float32), b1q], axis=1).astype(
        np.float32
    )
    return {
        "usb": np.ascontiguousarray(usb.astype(mm_np)),
        "cstm": np.ascontiguousarray(vsb.astype(mm_np)),
        "cstb": np.ascontiguousarray(cstb),
    }


def kernel(x, A, W1, b1, W2, b2, mode=MM_MODE, trace=False):
    _, mm_np = _mm_dtypes(mode)
    if mode == "bf16":
        x_np = _BF16_NP
    elif mode == "f16":
        x_np = np.float16
    else:
        x_np = np.float32
    # Host-side layout staging: per-core transpose to [F, B_CORE] so the
    # contraction dim is on partitions at DMA time (no on-chip transpose).
    x = np.asarray(x, np.float32).astype(x_np).reshape(N_CORES, B_CORE, F)
    weights = _host_tensors(A, W1, b1, W2, b2, mode)

    nc = build_bass(mode)
    in_maps = []
    for i in range(N_CORES):
        m = {"xs": np.ascontiguousarray(x[i].T)}
        m.update(weights)
        in_maps.append(m)

    res = run_bass_kernel_spmd(nc, in_maps, core_ids=list(range(N_CORES)), trace=trace)
    out = np.concatenate([r["out"] for r in res.results], axis=0)
    out = out.reshape(B, C, D).astype(np.float32)
    if trace:
        return out, res
    return out
